# revision 1
# baseline (speedup 1.0000x reference)
"""Trainium2 Bass kernel for nn_AberrationCorrectionModule.

Reference pipeline:
  1. psf_predictor: 3x conv3x3 (128->256->128->900) on aberration_features,
     softmax over 225 taps per channel -> psf
  2. deconv: 15x15 spatially-varying weighted sum over reflect-padded raw
  3. freq corrector: rfft2 -> conv3x3 stack (8->64->64->8) -> irfft2, added
  4. per-channel refinement: 4 independent 1->16->16->1 conv stacks
  5. out = clip(raw + corrected, 0, 1)

Distribution: 8 NeuronCores, H-sharded (32 rows/core), SPMD dispatches with
host gather between (FFT stage needs full-image mixing).
"""
import json
import sys

sys.path.insert(0, "/opt/trn_rl_repo")

import ml_dtypes
import numpy as np

import bass_rust
import concourse.bass as bass
import concourse.tile as tile
from concourse import mybir
from concourse.bass_utils import run_bass_kernel_spmd

F32 = mybir.dt.float32
BF16 = mybir.dt.bfloat16
AF = mybir.ActivationFunctionType
ALU = mybir.AluOpType
AX = mybir.AxisListType

N_CORES = 8
C, H, W = 4, 256, 256
ROWS = H // N_CORES  # 32
KK = 15
PAD = KK // 2  # 7
WP = W + 2  # 258
TAPS = [(dy, dx) for dy in (-1, 0, 1) for dx in (-1, 0, 1)]


def _bf(x):
    return np.asarray(x, dtype=ml_dtypes.bfloat16)


def mkap(base_ap, offset, pairs):
    a = base_ap.copy()
    a.offset = offset
    a.ap = bass_rust.VecI64Pair([list(p) for p in pairs])
    return a


def _split_multiwaits(raw: bytes) -> bytes:
    """Workaround: this walrus build rejects >1 sync wait per instruction.
    Move extra waits onto NoOp carriers inserted just before the instruction."""
    m = json.loads(raw)
    ctr = 0
    for fn in m["functions"]:
        for bb in fn.get("blocks", []):
            insts = bb.get("instructions")
            if not insts:
                continue
            out = []
            for inst in insts:
                si = inst.get("sync_info")
                ow = (si or {}).get("on_wait") or []
                if len(ow) > 1:
                    for w in ow[:-1]:
                        out.append({
                            "debug": inst.get("debug", 0),
                            "engine": inst["engine"],
                            "ins": [], "outs": [],
                            "name": f"wsplit_{ctr}",
                            "opcode": "NoOp",
                            "sync_info": {"on_update": [], "on_wait": [w]},
                        })
                        ctr += 1
                    si["on_wait"] = [ow[-1]]
                out.append(inst)
            bb["instructions"] = out
    return json.dumps(m).encode()


def patch_nc(nc):
    orig = nc.to_json_bytes
    nc.to_json_bytes = lambda: _split_multiwaits(orig())
    return nc


def nchunks(total, step):
    out, o = [], 0
    while o < total:
        out.append((o, min(step, total - o)))
        o += step
    return out


# ================================================================ kernel A
# conv grids are [128part, rows, 258] flattened on the free axis;
# real cols live at 1..257, cols 0/257 are zero padding.

def build_A():
    nc = bass.Bass(trn_type="TRN2", name="kernA")
    feat = nc.dram_tensor("feat", (128, 38 * 256), F32, kind="ExternalInput")
    raw46 = nc.dram_tensor("raw46", (C, 46, 270), BF16, kind="ExternalInput")
    w1 = nc.dram_tensor("w1", (128, 2 * 9 * 128), BF16, kind="ExternalInput")
    b1 = nc.dram_tensor("b1", (128, 2), F32, kind="ExternalInput")
    w2 = nc.dram_tensor("w2", (128, 2 * 9 * 128), BF16, kind="ExternalInput")
    b2 = nc.dram_tensor("b2", (128, 1), F32, kind="ExternalInput")
    w3 = nc.dram_tensor("w3", (128, 9 * 1024), BF16, kind="ExternalInput")
    b3 = nc.dram_tensor("b3", (128, 8), F32, kind="ExternalInput")
    m36 = nc.dram_tensor("m36", (128, 36), F32, kind="ExternalInput")
    m34 = nc.dram_tensor("m34", (128, 34), F32, kind="ExternalInput")
    corr = nc.dram_tensor("corr", (C, ROWS, W), F32, kind="ExternalOutput")

    NF36, NF34 = 36 * WP, 34 * WP

    with tile.TileContext(nc) as tc:
        with tc.tile_pool(name="cst", bufs=1) as cst, \
             tc.tile_pool(name="hp", bufs=1) as hp, \
             tc.tile_pool(name="psum", bufs=2, space="PSUM") as psp:
            w3t = cst.tile([128, 9 * 1024], BF16)
            nc.sync.dma_start(w3t[:], w3[:])
            b3t = cst.tile([128, 8], F32)
            nc.sync.dma_start(b3t[:], b3[:])
            b2t = cst.tile([128, 1], F32)
            nc.sync.dma_start(b2t[:], b2[:])
            m34t = cst.tile([128, 34], F32)
            nc.sync.dma_start(m34t[:], m34[:])
            ones = cst.tile([128, 1], BF16)
            nc.vector.memset(ones[:], 1.0)

            h2 = hp.tile([128, NF34], BF16)

            with tc.tile_pool(name="h1p", bufs=1) as h1p:
                h1 = [h1p.tile([128, NF36 + 8], BF16, name=f"h1_{m}", tag=f"h1_{m}") for m in range(2)]
                w2t = h1p.tile([128, 2 * 9 * 128], BF16)
                nc.sync.dma_start(w2t[:], w2[:])

                with tc.tile_pool(name="fp", bufs=1) as fp:
                    w1t = fp.tile([128, 2 * 9 * 128], BF16)
                    nc.sync.dma_start(w1t[:], w1[:])
                    b1t = fp.tile([128, 2], F32)
                    nc.sync.dma_start(b1t[:], b1[:])
                    m36t = fp.tile([128, 36], F32)
                    nc.sync.dma_start(m36t[:], m36[:])
                    ff = fp.tile([128, 38 * 256], F32)
                    nc.sync.dma_start(ff[:], feat[:])
                    fb = fp.tile([128, 38 * WP + 8], BF16)
                    nc.vector.memset(fb[:], 0.0)
                    nc.vector.tensor_copy(
                        fb[:, 1:1 + 38 * WP].rearrange(
                            "p (r c) -> p r c", r=38)[:, :, 1:257],
                        ff[:].rearrange("p (r c) -> p r c", r=38))

                    # conv1: 128 -> 256 (2 M chunks), taps-outer groups of 3
                    for m in range(2):
                        ch1 = nchunks(NF36, 512)
                        for g0 in range(0, len(ch1), 3):
                            grp = ch1[g0:g0 + 3]
                            pcs = [psp.tile([128, 512], F32, tag=f"pc{j}",
                                            name=f"c1_{m}_{g0}_{j}", bufs=1)
                                   for j in range(len(grp))]
                            for t, (dy, dx) in enumerate(TAPS):
                                base = (1 + dy) * WP + dx
                                for j, (n0, nl) in enumerate(grp):
                                    nc.tensor.matmul(
                                        pcs[j][:, :nl],
                                        lhsT=w1t[:, (m * 9 + t) * 128:(m * 9 + t + 1) * 128],
                                        rhs=fb[:, 1 + n0 + base:1 + n0 + base + nl],
                                        start=(t == 0), stop=(t == 8))
                            for j, (n0, nl) in enumerate(grp):
                                nc.scalar.activation(
                                    h1[m][:, 1 + n0:1 + n0 + nl], pcs[j][:, :nl],
                                    AF.Relu, bias=b1t[:, m:m + 1])
                        h3 = h1[m][:, 1:1 + NF36].rearrange("p (r c) -> p r c", r=36)
                        nc.vector.memset(h3[:, :, 0:1], 0.0)
                        nc.vector.memset(h3[:, :, 257:258], 0.0)
                        # zero out-of-image rows (only rows 0,1,34,35 can be OOI)
                        for r in (0, 1, 34, 35):
                            nc.vector.tensor_scalar_mul(
                                h3[:, r, :], h3[:, r, :], m36t[:, r:r + 1])

                # conv2: 256 -> 128 (2 K chunks), taps-outer groups of 3
                ch2 = nchunks(NF34, 512)
                for g0 in range(0, len(ch2), 3):
                    grp = ch2[g0:g0 + 3]
                    pcs = [psp.tile([128, 512], F32, tag=f"pc{j}",
                                    name=f"c2_{g0}_{j}", bufs=1)
                           for j in range(len(grp))]
                    ti = 0
                    for kc in range(2):
                        for t, (dy, dx) in enumerate(TAPS):
                            base = (1 + dy) * WP + dx
                            for j, (n0, nl) in enumerate(grp):
                                nc.tensor.matmul(
                                    pcs[j][:, :nl],
                                    lhsT=w2t[:, (kc * 9 + t) * 128:(kc * 9 + t + 1) * 128],
                                    rhs=h1[kc][:, 1 + n0 + base:1 + n0 + base + nl],
                                    start=(ti == 0), stop=(ti == 17))
                            ti += 1
                    for j, (n0, nl) in enumerate(grp):
                        nc.scalar.activation(
                            h2[:, n0:n0 + nl], pcs[j][:, :nl], AF.Relu, bias=b2t[:])
                h23 = h2[:].rearrange("p (r c) -> p r c", r=34)
                nc.vector.memset(h23[:, :, 0:1], 0.0)
                nc.vector.memset(h23[:, :, 257:258], 0.0)
                for r in (0, 33):
                    nc.vector.tensor_scalar_mul(
                        h23[:, r, :], h23[:, r, :], m34t[:, r:r + 1])

            # conv3 + softmax + deconv per (pixchunk, channel).
            # psf channels padded 900->1024: image channel c = M-chunks
            # {2c, 2c+1}; taps 0..224 real, 225..255 padded (bias -30).
            RPC = 8
            PCN = RPC * W  # 2048
            h2v = h2[:].rearrange("p (r q) -> p r q", r=34)
            with tc.tile_pool(name="ex", bufs=2) as exp_pool, \
                 tc.tile_pool(name="xp", bufs=2) as xpool, \
                 tc.tile_pool(name="scp", bufs=2) as scp, \
                 tc.tile_pool(name="dnp", bufs=2, space="DRAM") as dnp, \
                 tc.tile_pool(name="rbp", bufs=2) as rbp:
                for pc_i in range(ROWS // RPC):
                    r0 = pc_i * RPC
                    dnd = dnp.tile([C, 2 * PCN], F32, tag="dnd")
                    for c in range(C):
                        Ea = exp_pool.tile([128, PCN], BF16, tag="Ea")
                        Eb = exp_pool.tile([128, PCN], BF16, tag="Eb")
                        Pa = exp_pool.tile([128, PCN], BF16, tag="Pa")
                        Pb = exp_pool.tile([128, PCN], BF16, tag="Pb")
                        Xa = xpool.tile([128, PCN], BF16, tag="Xa")
                        Xb = xpool.tile([128, PCN], BF16, tag="Xb")
                        # patch strips: partition t = dy*15+dx, free = pixel
                        for dy in range(KK):
                            t0 = dy * KK
                            off = c * 46 * 270 + (r0 + dy) * 270
                            if t0 + KK <= 128:
                                nc.sync.dma_start(
                                    Xa[t0:t0 + KK, :],
                                    mkap(raw46[:], off, [[1, KK], [270, RPC], [1, W]]))
                            elif t0 >= 128:
                                nc.sync.dma_start(
                                    Xb[t0 - 128:t0 - 128 + KK, :],
                                    mkap(raw46[:], off, [[1, KK], [270, RPC], [1, W]]))
                            else:
                                n1 = 128 - t0
                                nc.sync.dma_start(
                                    Xa[t0:128, :],
                                    mkap(raw46[:], off, [[1, n1], [270, RPC], [1, W]]))
                                nc.sync.dma_start(
                                    Xb[0:KK - n1, :],
                                    mkap(raw46[:], off + n1,
                                         [[1, KK - n1], [270, RPC], [1, W]]))
                        # conv3 -> exp (bias fused into exp's activation)
                        for half, E in ((0, Ea), (1, Eb)):
                            mc = c * 2 + half
                            chunks = nchunks(PCN, 512)
                            pss = [psp.tile([128, 512], F32, tag=f"pc{j}",
                                            name=f"ps_{mc}_{j}", bufs=1)
                                   for j in range(len(chunks))]
                            for t, (dy, dx) in enumerate(TAPS):
                                for j, (s0, sl) in enumerate(chunks):
                                    rr = r0 + s0 // W + 1 + dy
                                    nc.tensor.matmul(
                                        pss[j][:, :sl],
                                        lhsT=w3t[:, t * 1024 + mc * 128:
                                                 t * 1024 + (mc + 1) * 128],
                                        rhs=h2v[:, rr:rr + 2, 1 + dx:257 + dx],
                                        start=(t == 0), stop=(t == 8))
                            for j, (s0, sl) in enumerate(chunks):
                                nc.scalar.activation(
                                    E[:, s0:s0 + sl], pss[j][:, :sl], AF.Exp,
                                    bias=b3t[:, mc:mc + 1])
                        # tap sums via ones-matmuls on PE (GPSIMD C-reduce
                        # is ~40us/op; PE does it in ~0.2us/chunk)
                        nc.vector.tensor_tensor(out=Pa[:, :], in0=Ea[:, :], in1=Xa[:, :], op=ALU.mult)
                        nc.vector.tensor_tensor(out=Pb[0:97, :], in0=Eb[0:97, :], in1=Xb[0:97, :], op=ALU.mult)
                        sc = scp.tile([1, 2 * PCN], F32, tag="sc")
                        da, na = sc[:, 0:PCN], sc[:, PCN:2 * PCN]
                        for s0, sl in nchunks(PCN, 512):
                            for dst, ta, tb in ((da, Ea, Eb), (na, Pa, Pb)):
                                pr = psp.tile([1, 512], F32, tag="pr", bufs=2)
                                nc.tensor.matmul(pr[:, :sl], lhsT=ones[:, :],
                                                 rhs=ta[:, s0:s0 + sl],
                                                 start=True, stop=False)
                                nc.tensor.matmul(pr[:, :sl], lhsT=ones[0:97, :],
                                                 rhs=tb[0:97, s0:s0 + sl],
                                                 start=False, stop=True)
                                nc.vector.tensor_copy(dst[:, s0:s0 + sl], pr[:, :sl])
                        nc.sync.dma_start(dnd[c, :], sc[:, :])
                    # reshape [1,2048]x2 per ch -> [128,64] so the divide
                    # runs on all 128 lanes instead of one
                    Dt = rbp.tile([128, 64], F32, tag="Dt")
                    Nt = rbp.tile([128, 64], F32, tag="Nt")
                    for c in range(C):
                        nc.sync.dma_start(
                            Dt[32 * c:32 * c + 32, :],
                            mkap(dnd[:], c * 2 * PCN, [[64, 32], [1, 64]]))
                        nc.sync.dma_start(
                            Nt[32 * c:32 * c + 32, :],
                            mkap(dnd[:], c * 2 * PCN + PCN, [[64, 32], [1, 64]]))
                    nc.vector.reciprocal(Dt[:], Dt[:])
                    nc.vector.tensor_tensor(out=Nt[:], in0=Nt[:], in1=Dt[:], op=ALU.mult)
                    nc.sync.dma_start(corr[:, r0:r0 + RPC, :], Nt[:])
    return nc




# ================================================================ kernel B1
# Forward rfft2 via DFT matmuls, replicated on every core; writes full fri.
# V[h,k] = sum_w x[h,w] Fw[w,k];  Y[k1,k] = sum_h Fh[k1,h] V[h,k]
# fri = [Yre(4ch), Yim(4ch)] as [8, 256, 129].

def build_B1():
    nc = bass.Bass(trn_type="TRN2", name="kernB1")
    corrT = nc.dram_tensor("corrT", (C, 256, 256), BF16, kind="ExternalInput")
    fwre = nc.dram_tensor("fwre", (256, 129), BF16, kind="ExternalInput")
    fwim = nc.dram_tensor("fwim", (256, 129), BF16, kind="ExternalInput")
    fhre = nc.dram_tensor("fhre", (256, 256), BF16, kind="ExternalInput")
    fhim = nc.dram_tensor("fhim", (256, 256), BF16, kind="ExternalInput")
    fhimn = nc.dram_tensor("fhimn", (256, 256), BF16, kind="ExternalInput")
    fri = nc.dram_tensor("fri", (8, 256, 129), F32, kind="ExternalOutput")

    with tile.TileContext(nc) as tc:
        with tc.tile_pool(name="cst", bufs=1) as cst, \
             tc.tile_pool(name="wk", bufs=2) as wk, \
             tc.tile_pool(name="ps", bufs=4, space="PSUM") as psp:
            fw = [cst.tile([128, 2 * 129], BF16, name=f"fw_{i}", tag=f"fw_{i}") for i in range(2)]
            for kc in range(2):
                nc.sync.dma_start(fw[kc][:, 0:129], fwre[kc * 128:(kc + 1) * 128, :])
                nc.sync.dma_start(fw[kc][:, 129:258], fwim[kc * 128:(kc + 1) * 128, :])
            fh = [cst.tile([128, 3 * 256], BF16, name=f"fh_{i}", tag=f"fh_{i}") for i in range(2)]
            for kc in range(2):
                nc.sync.dma_start(fh[kc][:, 0:256], fhre[kc * 128:(kc + 1) * 128, :])
                nc.sync.dma_start(fh[kc][:, 256:512], fhim[kc * 128:(kc + 1) * 128, :])
                nc.sync.dma_start(fh[kc][:, 512:768], fhimn[kc * 128:(kc + 1) * 128, :])
            for c in range(C):
                xT = [wk.tile([128, 256], BF16, name=f"xT{i}", tag=f"xT{i}") for i in range(2)]
                for kc in range(2):
                    nc.sync.dma_start(xT[kc][:], corrT[c, kc * 128:(kc + 1) * 128, :])
                V = [wk.tile([128, 2 * 129], BF16, name=f"V{i}", tag=f"V{i}") for i in range(2)]
                for mc in range(2):      # output h chunk
                    for ri in range(2):  # re / im
                        pv = psp.tile([128, 129], F32, tag="pv")
                        for kc in range(2):
                            nc.tensor.matmul(
                                pv[:, :],
                                lhsT=xT[kc][:, mc * 128:(mc + 1) * 128],
                                rhs=fw[kc][:, ri * 129:(ri + 1) * 129],
                                start=(kc == 0), stop=(kc == 1))
                        nc.vector.tensor_copy(V[mc][:, ri * 129:(ri + 1) * 129], pv[:, :])
                # Y: for re out: FhRe@Vre + FhImNeg@Vim ; im out: FhIm@Vre + FhRe@Vim
                for mc in range(2):      # k1 chunk
                    for ri in range(2):  # re / im output
                        py = psp.tile([128, 129], F32, tag="pv")
                        for kc in range(2):
                            if ri == 0:
                                t1, t2 = 0, 512   # re, imneg
                            else:
                                t1, t2 = 256, 0   # im, re
                            nc.tensor.matmul(
                                py[:, :],
                                lhsT=fh[kc][:, t1 + mc * 128:t1 + (mc + 1) * 128],
                                rhs=V[kc][:, 0:129],
                                start=(kc == 0), stop=False)
                            nc.tensor.matmul(
                                py[:, :],
                                lhsT=fh[kc][:, t2 + mc * 128:t2 + (mc + 1) * 128],
                                rhs=V[kc][:, 129:258],
                                start=False, stop=(kc == 1))
                        ys = wk.tile([128, 129], F32, tag="ys")
                        nc.scalar.activation(ys[:], py[:], AF.Copy)
                        nc.sync.dma_start(
                            fri[ri * 4 + c, mc * 128:(mc + 1) * 128, :], ys[:])
    return nc


# ================================================================ kernel B2
# freq conv stack on fri slab (38 rows, ch-major) + partial inverse fft.
WF = 131  # 129 + 2 pad cols

def build_B2():
    nc = bass.Bass(trn_type="TRN2", name="kernB2")
    fri = nc.dram_tensor("fri", (8, 38 * WF), BF16, kind="ExternalInput")
    gw1 = nc.dram_tensor("gw1", (8, 9 * 64), BF16, kind="ExternalInput")
    gb1 = nc.dram_tensor("gb1", (64, 1), F32, kind="ExternalInput")
    gw2 = nc.dram_tensor("gw2", (64, 9 * 64), BF16, kind="ExternalInput")
    gb2 = nc.dram_tensor("gb2", (64, 1), F32, kind="ExternalInput")
    gw3 = nc.dram_tensor("gw3", (64, 9 * 8), BF16, kind="ExternalInput")
    gb3 = nc.dram_tensor("gb3", (8, 1), F32, kind="ExternalInput")
    mf36 = nc.dram_tensor("mf36", (64, 36), F32, kind="ExternalInput")
    mf34 = nc.dram_tensor("mf34", (64, 34), F32, kind="ExternalInput")
    iwre = nc.dram_tensor("iwre", (129, 256), BF16, kind="ExternalInput")
    iwim = nc.dram_tensor("iwim", (129, 256), BF16, kind="ExternalInput")
    iwimn = nc.dram_tensor("iwimn", (129, 256), BF16, kind="ExternalInput")
    ihre = nc.dram_tensor("ihre", (32, 256), BF16, kind="ExternalInput")
    ihimn = nc.dram_tensor("ihimn", (32, 256), BF16, kind="ExternalInput")
    zp = nc.dram_tensor("zp", (C, 256, 256), BF16, kind="ExternalOutput")

    N36, N34, N32 = 36 * WF, 34 * WF, 32 * WF

    with tile.TileContext(nc) as tc:
        with tc.tile_pool(name="cst", bufs=1) as cst, \
             tc.tile_pool(name="gp", bufs=1) as gp, \
             tc.tile_pool(name="ps", bufs=4, space="PSUM") as psp:
            w1t = cst.tile([8, 9 * 64], BF16)
            nc.sync.dma_start(w1t[:], gw1[:])
            w2t = cst.tile([64, 9 * 64], BF16)
            nc.sync.dma_start(w2t[:], gw2[:])
            w3t = cst.tile([64, 9 * 8], BF16)
            nc.sync.dma_start(w3t[:], gw3[:])
            b1t = cst.tile([64, 1], F32)
            nc.sync.dma_start(b1t[:], gb1[:])
            b2t = cst.tile([64, 1], F32)
            nc.sync.dma_start(b2t[:], gb2[:])
            b3t = cst.tile([8, 1], F32)
            nc.sync.dma_start(b3t[:], gb3[:])
            m36t = cst.tile([64, 36], F32)
            nc.sync.dma_start(m36t[:], mf36[:])
            m34t = cst.tile([64, 34], F32)
            nc.sync.dma_start(m34t[:], mf34[:])
            iw = cst.tile([128, 3 * 256], BF16)
            iwb = cst.tile([1, 3 * 256], BF16)
            for j, srcm in enumerate((iwre, iwim, iwimn)):
                nc.sync.dma_start(iw[:, j * 256:(j + 1) * 256], srcm[0:128, :])
                nc.sync.dma_start(iwb[:, j * 256:(j + 1) * 256], srcm[128:129, :])
            ih = cst.tile([32, 2 * 256], BF16)
            nc.sync.dma_start(ih[:, 0:256], ihre[:])
            nc.sync.dma_start(ih[:, 256:512], ihimn[:])

            ft = gp.tile([8, 1 + 38 * WF + 4], BF16)
            nc.sync.dma_start(ft[:, 1:1 + 38 * WF], fri[:, :])
            g1 = gp.tile([64, 1 + N36 + 4], BF16)
            g2 = gp.tile([64, 1 + N34 + 4], BF16)
            g3 = gp.tile([8, N32], F32)

            for n0, nl in nchunks(N36, 512):
                pc = psp.tile([64, 512], F32, tag="pg")
                for t, (dy, dx) in enumerate(TAPS):
                    base = (1 + dy) * WF + dx
                    nc.tensor.matmul(
                        pc[:, :nl],
                        lhsT=w1t[:, t * 64:(t + 1) * 64],
                        rhs=ft[:, 1 + n0 + base:1 + n0 + base + nl],
                        start=(t == 0), stop=(t == 8))
                nc.scalar.activation(g1[:, 1 + n0:1 + n0 + nl], pc[:, :nl],
                                     AF.Relu, bias=b1t[:])
            g1v = g1[:, 1:1 + N36].rearrange("p (r q) -> p r q", r=36)
            nc.vector.memset(g1v[:, :, 0:1], 0.0)
            nc.vector.memset(g1v[:, :, 130:131], 0.0)
            for r in (0, 1, 34, 35):
                nc.vector.tensor_scalar_mul(g1v[:, r, :], g1v[:, r, :],
                                            m36t[:, r:r + 1])
            for n0, nl in nchunks(N34, 512):
                pc = psp.tile([64, 512], F32, tag="pg")
                for t, (dy, dx) in enumerate(TAPS):
                    base = (1 + dy) * WF + dx
                    nc.tensor.matmul(
                        pc[:, :nl],
                        lhsT=w2t[:, t * 64:(t + 1) * 64],
                        rhs=g1[:, 1 + n0 + base:1 + n0 + base + nl],
                        start=(t == 0), stop=(t == 8))
                nc.scalar.activation(g2[:, 1 + n0:1 + n0 + nl], pc[:, :nl],
                                     AF.Relu, bias=b2t[:])
            g2v = g2[:, 1:1 + N34].rearrange("p (r q) -> p r q", r=34)
            nc.vector.memset(g2v[:, :, 0:1], 0.0)
            nc.vector.memset(g2v[:, :, 130:131], 0.0)
            for r in (0, 33):
                nc.vector.tensor_scalar_mul(g2v[:, r, :], g2v[:, r, :],
                                            m34t[:, r:r + 1])
            for n0, nl in nchunks(N32, 512):
                pc = psp.tile([8, 512], F32, tag="pg")
                for t, (dy, dx) in enumerate(TAPS):
                    base = (1 + dy) * WF + dx
                    nc.tensor.matmul(
                        pc[:, :nl],
                        lhsT=w3t[:, t * 8:(t + 1) * 8],
                        rhs=g2[:, 1 + n0 + base:1 + n0 + base + nl],
                        start=(t == 0), stop=(t == 8))
                nc.scalar.activation(g3[:, n0:n0 + nl], pc[:, :nl],
                                     AF.Copy, bias=0.0)
            # add bias gb3 separately (Copy cannot take AP bias)
            nc.vector.tensor_scalar(out=g3[:], in0=g3[:], scalar1=b3t[:],
                                    scalar2=None, op0=ALU.add)

            # shuffle CF to k-major [128+1, 32] per (c, re/im) via DRAM scratch
            with tc.tile_pool(name="dsc", bufs=1, space="DRAM") as dsc, \
                 tc.tile_pool(name="inv", bufs=2) as inv:
                gdr = dsc.tile([8, N32], F32)
                nc.sync.dma_start(gdr[:], g3[:])
                for c in range(C):
                    cfa = inv.tile([128, 2 * 32], F32, tag="cfa")
                    cfb = inv.tile([1, 2 * 32], F32, tag="cfb")
                    for ri in range(2):
                        base = (ri * 4 + c) * N32
                        nc.sync.dma_start(
                            cfa[:, ri * 32:(ri + 1) * 32],
                            mkap(gdr[:], base + 1, [[1, 128], [WF, 32]]))
                        nc.sync.dma_start(
                            cfb[:, ri * 32:(ri + 1) * 32],
                            mkap(gdr[:], base + 129, [[1, 1], [WF, 32]]))
                    cfab = inv.tile([128, 2 * 32], BF16, tag="cfab")
                    cfbb = inv.tile([1, 2 * 32], BF16, tag="cfbb")
                    nc.vector.tensor_copy(cfab[:], cfa[:])
                    nc.vector.tensor_copy(cfbb[:], cfb[:])
                    # B = CF @ iFw^T : [32 k1, 256 w] complex
                    Bt = inv.tile([32, 2 * 256], BF16, tag="Bt")
                    for ri in range(2):
                        pb = psp.tile([32, 256], F32, tag="pb", bufs=2)
                        if ri == 0:
                            j1, j2 = 0, 2   # re*re + im*imneg
                        else:
                            j1, j2 = 1, 0   # re*im + im*re
                        nc.tensor.matmul(pb[:], lhsT=cfab[:, 0:32],
                                         rhs=iw[:, j1 * 256:(j1 + 1) * 256],
                                         start=True, stop=False)
                        nc.tensor.matmul(pb[:], lhsT=cfbb[:, 0:32],
                                         rhs=iwb[:, j1 * 256:(j1 + 1) * 256],
                                         start=False, stop=False)
                        nc.tensor.matmul(pb[:], lhsT=cfab[:, 32:64],
                                         rhs=iw[:, j2 * 256:(j2 + 1) * 256],
                                         start=False, stop=False)
                        nc.tensor.matmul(pb[:], lhsT=cfbb[:, 32:64],
                                         rhs=iwb[:, j2 * 256:(j2 + 1) * 256],
                                         start=False, stop=True)
                        nc.vector.tensor_copy(Bt[:, ri * 256:(ri + 1) * 256], pb[:])
                    # z_part = ihre.T @ Bre + ihimn.T @ Bim : [256 h, 256 w]
                    for mc in range(2):
                        pz = psp.tile([128, 256], F32, tag="pz", bufs=2)
                        nc.tensor.matmul(pz[:], lhsT=ih[:, mc * 128:(mc + 1) * 128],
                                         rhs=Bt[:, 0:256], start=True, stop=False)
                        nc.tensor.matmul(pz[:],
                                         lhsT=ih[:, 256 + mc * 128:256 + (mc + 1) * 128],
                                         rhs=Bt[:, 256:512], start=False, stop=True)
                        zs = inv.tile([128, 256], BF16, tag="zs")
                        nc.vector.tensor_copy(zs[:], pz[:])
                        nc.sync.dma_start(zp[c, mc * 128:(mc + 1) * 128, :], zs[:])
    return nc


# ================================================================ kernel C
# per-channel refinement: block-diagonal batched convs 4->64->64->4.

def build_C():
    nc = bass.Bass(trn_type="TRN2", name="kernC")
    u = nc.dram_tensor("u", (C, 38 * WP), BF16, kind="ExternalInput")
    raw32 = nc.dram_tensor("raw32", (C, ROWS * W), F32, kind="ExternalInput")
    cw1 = nc.dram_tensor("cw1", (C, 9 * 64), BF16, kind="ExternalInput")
    cb1 = nc.dram_tensor("cb1", (64, 1), F32, kind="ExternalInput")
    cw2 = nc.dram_tensor("cw2", (64, 9 * 64), BF16, kind="ExternalInput")
    cb2 = nc.dram_tensor("cb2", (64, 1), F32, kind="ExternalInput")
    cw3 = nc.dram_tensor("cw3", (64, 9 * 4), BF16, kind="ExternalInput")
    cb3 = nc.dram_tensor("cb3", (4, 1), F32, kind="ExternalInput")
    mr36 = nc.dram_tensor("mr36", (64, 36), F32, kind="ExternalInput")
    mr34 = nc.dram_tensor("mr34", (64, 34), F32, kind="ExternalInput")
    fin = nc.dram_tensor("fin", (C, ROWS, W), F32, kind="ExternalOutput")

    N36, N34, N32 = 36 * WP, 34 * WP, 32 * WP

    def conv_taps_outer(pool_ps, lhsw, rhsrc, dstact, bias, Ntot, Kp, Mp, relu,
                        group=1):
        """taps-outer grouped conv: lhsw(t)->lhsT AP, rhsrc(t, n0, nl)->rhs AP,
        dstact(n0, nl, psum) consumes."""
        chunks = nchunks(Ntot, 512)
        for g0 in range(0, len(chunks), group):
            grp = chunks[g0:g0 + group]
            pss = [pool_ps.tile([Mp, 512], F32, tag=f"cg{j}", name=f"cg_{g0}_{j}",
                                bufs=6) for j in range(len(grp))]
            for t in range(9):
                for j, (n0, nl) in enumerate(grp):
                    nc.tensor.matmul(pss[j][:, :nl], lhsT=lhsw(t),
                                     rhs=rhsrc(t, n0, nl),
                                     start=(t == 0), stop=(t == 8))
            for j, (n0, nl) in enumerate(grp):
                dstact(n0, nl, pss[j])

    with tile.TileContext(nc) as tc:
        with tc.tile_pool(name="cst", bufs=1) as cst, \
             tc.tile_pool(name="gp", bufs=1) as gp, \
             tc.tile_pool(name="ps", bufs=1, space="PSUM") as psp:
            w1t = cst.tile([C, 9 * 64], BF16)
            nc.sync.dma_start(w1t[:], cw1[:])
            w2t = cst.tile([64, 9 * 64], BF16)
            nc.sync.dma_start(w2t[:], cw2[:])
            w3t = cst.tile([64, 9 * 4], BF16)
            nc.sync.dma_start(w3t[:], cw3[:])
            b1t = cst.tile([64, 1], F32)
            nc.sync.dma_start(b1t[:], cb1[:])
            b2t = cst.tile([64, 1], F32)
            nc.sync.dma_start(b2t[:], cb2[:])
            b3t = cst.tile([C, 1], F32)
            nc.sync.dma_start(b3t[:], cb3[:])
            m36t = cst.tile([64, 36], F32)
            nc.sync.dma_start(m36t[:], mr36[:])
            m34t = cst.tile([64, 34], F32)
            nc.sync.dma_start(m34t[:], mr34[:])

            ut = gp.tile([C, 1 + 38 * WP + 4], BF16)
            nc.sync.dma_start(ut[:, 1:1 + 38 * WP], u[:])
            r1 = gp.tile([64, 1 + N36 + 4], BF16)
            r2 = gp.tile([64, 1 + N34 + 4], BF16)

            conv_taps_outer(
                psp,
                lambda t: w1t[:, t * 64:(t + 1) * 64],
                lambda t, n0, nl: ut[:, 1 + n0 + (1 + TAPS[t][0]) * WP + TAPS[t][1]:
                                     1 + n0 + (1 + TAPS[t][0]) * WP + TAPS[t][1] + nl],
                lambda n0, nl, ps: nc.scalar.activation(
                    r1[:, 1 + n0:1 + n0 + nl], ps[:, :nl], AF.Relu, bias=b1t[:]),
                b1t, N36, 64, 64, True)
            r1v = r1[:, 1:1 + N36].rearrange("p (r q) -> p r q", r=36)
            nc.vector.memset(r1v[:, :, 0:1], 0.0)
            nc.vector.memset(r1v[:, :, 257:258], 0.0)
            for r in (0, 1, 34, 35):
                nc.vector.tensor_scalar_mul(r1v[:, r, :], r1v[:, r, :],
                                            m36t[:, r:r + 1])

            conv_taps_outer(
                psp,
                lambda t: w2t[:, t * 64:(t + 1) * 64],
                lambda t, n0, nl: r1[:, 1 + n0 + (1 + TAPS[t][0]) * WP + TAPS[t][1]:
                                     1 + n0 + (1 + TAPS[t][0]) * WP + TAPS[t][1] + nl],
                lambda n0, nl, ps: nc.scalar.activation(
                    r2[:, 1 + n0:1 + n0 + nl], ps[:, :nl], AF.Relu, bias=b2t[:]),
                b2t, N34, 64, 64, True)
            r2v = r2[:, 1:1 + N34].rearrange("p (r q) -> p r q", r=34)
            nc.vector.memset(r2v[:, :, 0:1], 0.0)
            nc.vector.memset(r2v[:, :, 257:258], 0.0)
            for r in (0, 33):
                nc.vector.tensor_scalar_mul(r2v[:, r, :], r2v[:, r, :],
                                            m34t[:, r:r + 1])

            with tc.tile_pool(name="fo", bufs=1) as fo:
                rawt = fo.tile([C, ROWS * W], F32)
                nc.sync.dma_start(rawt[:], raw32[:])
                r3 = fo.tile([C, N32], F32)
                conv_taps_outer(
                    psp,
                    lambda t: w3t[:, t * 4:(t + 1) * 4],
                    lambda t, n0, nl: r2[:, 1 + n0 + (1 + TAPS[t][0]) * WP + TAPS[t][1]:
                                         1 + n0 + (1 + TAPS[t][0]) * WP + TAPS[t][1] + nl],
                    lambda n0, nl, ps: nc.scalar.activation(
                        r3[:, n0:n0 + nl], ps[:, :nl], AF.Copy),
                    b3t, N32, 64, C, False)
                r3v = r3[:].rearrange("p (r q) -> p r q", r=32)[:, :, 1:257]
                rv = rawt[:].rearrange("p (r q) -> p r q", r=32)
                nc.vector.tensor_scalar(out=r3v, in0=r3v, scalar1=b3t[:],
                                        scalar2=None, op0=ALU.add)
                nc.vector.tensor_tensor(out=r3v, in0=r3v, in1=rv, op=ALU.add)
                nc.vector.tensor_scalar(out=r3v, in0=r3v, scalar1=0.0,
                                        scalar2=1.0, op0=ALU.max, op1=ALU.min)
                nc.sync.dma_start(fin[:, :, :], r3v)
    return nc


_CACHE = {}


def _prep_A(raw, feat, pw1, pb1, pw2, pb2, pw3, pb3):
    from einops import rearrange as rr
    w1h = _bf(rr(pw1, "(m co) ci dy dx -> ci (m dy dx co)", m=2))
    b1h = np.ascontiguousarray(pb1.reshape(2, 128).T)
    w2h = _bf(rr(pw2, "co (kc cip) dy dx -> cip (kc dy dx co)", kc=2))
    b2h = pb2.reshape(128, 1).astype(np.float32)
    w3h = np.zeros((128, 9, 1024), np.float32)
    b3f = np.full((1024,), -30.0, np.float32)
    for c in range(C):
        w3h[:, :, c * 256:c * 256 + 225] = rr(
            pw3[c * 225:(c + 1) * 225], "o ci dy dx -> ci (dy dx) o")
        b3f[c * 256:c * 256 + 225] = pb3[c * 225:(c + 1) * 225]
    w3h = _bf(w3h.reshape(128, 9 * 1024))
    b3h = np.ascontiguousarray(b3f.reshape(8, 128).T)

    xpad = np.pad(raw, ((0, 0), (PAD, PAD), (PAD, PAD)), mode="reflect")
    featp = np.pad(feat, ((0, 0), (3, 3), (0, 0)))

    ins = []
    for i in range(N_CORES):
        r0 = i * ROWS
        m36 = np.array([1.0 if 0 <= r0 - 2 + r < H else 0.0
                        for r in range(36)], np.float32)
        m34 = np.array([1.0 if 0 <= r0 - 1 + r < H else 0.0
                        for r in range(34)], np.float32)
        ins.append({
            "feat": np.ascontiguousarray(
                featp[:, r0:r0 + 38, :]).reshape(128, 38 * 256),
            "raw46": _bf(np.ascontiguousarray(xpad[:, r0:r0 + 46, :])),
            "w1": w1h, "b1": b1h, "w2": w2h, "b2": b2h,
            "w3": w3h, "b3": b3h,
            "m36": np.ascontiguousarray(np.broadcast_to(m36, (128, 36))),
            "m34": np.ascontiguousarray(np.broadcast_to(m34, (128, 34))),
        })
    return ins


def run_A(raw, feat, pw1, pb1, pw2, pb2, pw3, pb3, trace=False):
    if "A" not in _CACHE:
        _CACHE["A"] = patch_nc(build_A())
    ins = _prep_A(raw, feat, pw1, pb1, pw2, pb2, pw3, pb3)
    res = run_bass_kernel_spmd(_CACHE["A"], ins, core_ids=list(range(N_CORES)),
                               trace=trace)
    corr = np.concatenate([res.results[i]["corr"] for i in range(N_CORES)], axis=1)
    return corr, res


def _dft_mats():
    k = np.arange(129)
    w = np.arange(256)
    th = 2 * np.pi * np.outer(w, k) / 256.0          # [256, 129]
    fwre = _bf(np.cos(th) / 16.0)
    fwim = _bf(-np.sin(th) / 16.0)
    h = np.arange(256)
    k1 = np.arange(256)
    th2 = 2 * np.pi * np.outer(h, k1) / 256.0        # [256h, 256k1]
    fhre = _bf(np.cos(th2) / 16.0)
    fhim = _bf(-np.sin(th2) / 16.0)
    fhimn = _bf(np.sin(th2) / 16.0)
    ck = np.where((k == 0) | (k == 128), 1.0, 2.0)
    th3 = 2 * np.pi * np.outer(k, w) / 256.0         # [129k, 256w]
    iwre = _bf(ck[:, None] * np.cos(th3) / 16.0)
    iwim = _bf(ck[:, None] * np.sin(th3) / 16.0)
    iwimn = _bf(-ck[:, None] * np.sin(th3) / 16.0)
    return fwre, fwim, fhre, fhim, fhimn, iwre, iwim, iwimn


def run_B1(corr1, trace=False):
    if "B1" not in _CACHE:
        _CACHE["B1"] = patch_nc(build_B1())
    fwre, fwim, fhre, fhim, fhimn, _, _, _ = _dft_mats()
    corrT = _bf(np.ascontiguousarray(corr1.transpose(0, 2, 1)))
    inm = {"corrT": corrT, "fwre": fwre, "fwim": fwim,
           "fhre": fhre, "fhim": fhim, "fhimn": fhimn}
    res = run_bass_kernel_spmd(_CACHE["B1"], [inm] * N_CORES,
                               core_ids=list(range(N_CORES)), trace=trace)
    return res.results[0]["fri"], res


def run_B2(fri_full, fw1, fb1, fw2, fb2, fw3, fb3, trace=False):
    from einops import rearrange as rr
    if "B2" not in _CACHE:
        _CACHE["B2"] = patch_nc(build_B2())
    _, _, _, _, _, iwre, iwim, iwimn = _dft_mats()
    gw1 = _bf(rr(fw1, "co ci dy dx -> ci (dy dx co)"))
    gw2 = _bf(rr(fw2, "co ci dy dx -> ci (dy dx co)"))
    gw3 = _bf(rr(fw3, "co ci dy dx -> ci (dy dx co)"))
    gb1 = fb1.reshape(64, 1).astype(np.float32)
    gb2 = fb2.reshape(64, 1).astype(np.float32)
    gb3 = fb3.reshape(8, 1).astype(np.float32)
    h = np.arange(256)
    ins = []
    for i in range(N_CORES):
        r0 = i * ROWS
        slab = np.zeros((8, 38, WF), np.float32)
        lo, hi = max(0, r0 - 3), min(256, r0 + 35)
        slab[:, lo - (r0 - 3):hi - (r0 - 3), 1:130] = fri_full[:, lo:hi, :]
        m36 = np.array([1.0 if 0 <= r0 - 2 + r < 256 else 0.0
                        for r in range(36)], np.float32)
        m34 = np.array([1.0 if 0 <= r0 - 1 + r < 256 else 0.0
                        for r in range(34)], np.float32)
        k1s = np.arange(r0, r0 + 32)
        th = 2 * np.pi * np.outer(k1s, h) / 256.0   # [32 k1, 256 h]
        ins.append({
            "fri": _bf(slab.reshape(8, 38 * WF)),
            "gw1": gw1, "gb1": gb1, "gw2": gw2, "gb2": gb2,
            "gw3": gw3, "gb3": gb3,
            "mf36": np.ascontiguousarray(np.broadcast_to(m36, (64, 36))),
            "mf34": np.ascontiguousarray(np.broadcast_to(m34, (64, 34))),
            "iwre": iwre, "iwim": iwim, "iwimn": iwimn,
            "ihre": _bf(np.cos(th) / 16.0),
            "ihimn": _bf(-np.sin(th) / 16.0),
        })
    res = run_bass_kernel_spmd(_CACHE["B2"], ins, core_ids=list(range(N_CORES)),
                               trace=trace)
    z = np.zeros((C, 256, 256), np.float32)
    for i in range(N_CORES):
        z += res.results[i]["zp"].astype(np.float32)
    return z, res


def run_C(corr2, raw, cw1, cb1, cw2, cb2, cw3, cb3, trace=False):
    if "C" not in _CACHE:
        _CACHE["C"] = patch_nc(build_C())
    w1h = np.zeros((4, 9, 64), np.float32)
    w2h = np.zeros((64, 9, 64), np.float32)
    w3h = np.zeros((64, 9, 4), np.float32)
    for c in range(C):
        for t, (dy, dx) in enumerate([(a, b) for a in range(3) for b in range(3)]):
            w1h[c, t, c * 16:(c + 1) * 16] = cw1[c, :, 0, dy, dx]
            w2h[c * 16:(c + 1) * 16, t, c * 16:(c + 1) * 16] = cw2[c, :, :, dy, dx].T
            w3h[c * 16:(c + 1) * 16, t, c] = cw3[c, 0, :, dy, dx]
    b1h = cb1.reshape(64, 1).astype(np.float32)
    b2h = cb2.reshape(64, 1).astype(np.float32)
    b3h = cb3.reshape(4, 1).astype(np.float32)
    ins = []
    for i in range(N_CORES):
        r0 = i * ROWS
        slab = np.zeros((C, 38, WP), np.float32)
        lo, hi = max(0, r0 - 3), min(256, r0 + 35)
        slab[:, lo - (r0 - 3):hi - (r0 - 3), 1:257] = corr2[:, lo:hi, :]
        m36 = np.array([1.0 if 0 <= r0 - 2 + r < 256 else 0.0
                        for r in range(36)], np.float32)
        m34 = np.array([1.0 if 0 <= r0 - 1 + r < 256 else 0.0
                        for r in range(34)], np.float32)
        ins.append({
            "u": _bf(slab.reshape(C, 38 * WP)),
            "raw32": np.ascontiguousarray(
                raw[:, r0:r0 + ROWS, :]).reshape(C, ROWS * W),
            "cw1": _bf(w1h.reshape(4, 9 * 64)),
            "cb1": b1h,
            "cw2": _bf(w2h.reshape(64, 9 * 64)),
            "cb2": b2h,
            "cw3": _bf(w3h.reshape(64, 9 * 4)),
            "cb3": b3h,
            "mr36": np.ascontiguousarray(np.broadcast_to(m36, (64, 36))),
            "mr34": np.ascontiguousarray(np.broadcast_to(m34, (64, 34))),
        })
    res = run_bass_kernel_spmd(_CACHE["C"], ins, core_ids=list(range(N_CORES)),
                               trace=trace)
    fin = np.concatenate([res.results[i]["fin"] for i in range(N_CORES)], axis=1)
    return fin, res


def kernel(**inputs):
    inputs = {k: np.asarray(v, dtype=np.float32) for k, v in inputs.items()}
    raw = inputs["raw_image"][0]
    feat = inputs["aberration_features"][0]
    corr1, _ = run_A(raw, feat,
                     inputs["pw1"], inputs["pb1"], inputs["pw2"], inputs["pb2"],
                     inputs["pw3"], inputs["pb3"])
    fri_full, _ = run_B1(corr1)
    z, _ = run_B2(fri_full, inputs["fw1"], inputs["fb1"], inputs["fw2"],
                  inputs["fb2"], inputs["fw3"], inputs["fb3"])
    corr2 = corr1 + z
    fin, _ = run_C(corr2, raw, inputs["cw1"], inputs["cb1"], inputs["cw2"],
                   inputs["cb2"], inputs["cw3"], inputs["cb3"])
    return fin[None].astype(np.float32)


